# revision 22
# baseline (speedup 1.0000x reference)
"""Trainium2 Bass kernel for nn_AsrModel (2-layer GRU encoder, 2-layer GRU
teacher-forced decoder, linear head + argmax).

Strategy:
  - Data-parallel over batch: B=1024 sharded 8 ways -> B_local=128 per core.
  - Hidden-major layout on chip: state tiles are [1+H, B_local] = [101, 128]
    with row 0 == 1.0 (the "ones row") so biases ride in as an extra
    contraction row of the gate matmuls.
  - Encoder (500 steps) runs in fp16 (weights/state/gates; PSUM fp32):
    fp16 matmul is 1 cyc/row vs fp32's 4, and DVE elementwise gets 2x mode.
  - Decoder (79 steps) runs in fp32 to keep logits accurate enough that the
    argmax output matches the reference (validated on host: fp16 encoder +
    fp32 decoder -> zero argmax flips, logits rel err ~1e-4).
  - The two layers of each phase are staggered by one time step - two
    independent dependency chains that hide per-op latency.
  - Per layer-step: 6 matmuls into PSUM (ih+hh per gate, biases via ones
    row), one merged Sigmoid over [100,256] (r|z), DVE gate arithmetic,
    Tanh, and an in-place h update.  h - n runs on GpSimd to offload DVE.
  - Final linear: lhsT = [lin_b; lin_W.T] (constant stationary), rhs = the
    decoder-l1 state tile -> logitsT [64, 128] in PSUM, copied out via
    ScalarE and DMA'd per step.
  - Host side: shard/transpose/pack inputs, embedding gather, final
    argmax + reshape (cheap numpy; HW time is the NEFF execution).
"""

import sys

sys.path.insert(0, "/opt/trn_rl_repo")

import numpy as np

from concourse import bass, bacc, tile
import concourse.mybir as mybir
from concourse.bass_utils import run_bass_kernel_spmd
from concourse import dve_ops
from concourse.dve_spec import Spec, Src0, Src1, lower as dve_lower
from concourse.dve_uop import DveOpSpec


def _register_gru_ns():
    """Custom fused DVE op: out = in0 - in0*in1 = n*(1-z).
    Saves one tensor_tensor on the critical h-update path."""
    if "GRU_NS" in dve_ops._SUB_OPCODE_FOR_NAME:
        return dve_ops._SUB_OPCODE_FOR_NAME and [
            o for o in dve_ops.OPS if o.name == "GRU_NS"][0]
    spec = Spec(
        body=Src0 - Src0 * Src1,
        reference=lambda in0, in1, s0, s1, imm2: in0.astype(np.float32)
        - in0.astype(np.float32) * in1.astype(np.float32),
    )
    opcode = dve_ops._CUSTOM_DVE_ROW_BASE + len(dve_ops.OPS)
    shas = {}
    for ver in ("v3", "v4"):
        uops = dve_lower(spec, ver=ver)
        shas[ver] = DveOpSpec(
            name="GRU_NS", opcode=opcode, uops=uops,
            rd1_en=True).sha(ver)
    op = dve_ops.DveOp("GRU_NS", spec, subdim=False, uops_sha=shas)
    dve_ops.OPS.append(op)
    dve_ops._SUB_OPCODE_FOR_NAME["GRU_NS"] = opcode
    return op


GRU_NS = _register_gru_ns()

T, B, E, H, P, D = 500, 1024, 26, 100, 80, 64
NCORES = 8
BL = B // NCORES  # 128
PD = P - 1  # decoder steps actually needed (logits[:-1])

F16 = mybir.dt.float16
F32 = mybir.dt.float32
AF = mybir.ActivationFunctionType
OP = mybir.AluOpType

USE_GPSIMD = True

_CACHE = {}


def _pack_w(Wih, bih, np_dt):
    """lhsT for gate matmuls, one [K,128] block per gate packed along the
    free dim -> [K_in+1, 384].  Rows = [W.T ; bias] matching rhs layouts
    [x ; ones] / [h ; ones] — the ones row is always LAST (engine access
    patterns must start at partition 0)."""
    K_in = Wih.shape[1]
    out = np.zeros((K_in + 1, 3 * 128), np.float32)
    for g in range(3):
        out[:K_in, g * 128:g * 128 + H] = Wih[g * H:(g + 1) * H, :].T
        out[K_in, g * 128:g * 128 + H] = bih[g * H:(g + 1) * H]
    return out.astype(np_dt)


def _prep_seq(x_tbe, np_dt):
    """[T, BL, E] -> [E+1, T*BL] with last row = 1.0 (ones row)."""
    Tn = x_tbe.shape[0]
    out = np.ones((E + 1, Tn * BL), np.float32)
    out[:E, :] = x_tbe.transpose(2, 0, 1).reshape(E, Tn * BL)
    return out.astype(np_dt)


def _gru_layer_steps(nc, tc, pools, cfg, t):
    """Emit one step of one GRU layer.

    h' = n*(1-z) + z*h with:
      - gate pre-activations via PSUM matmuls (biases ride the ones row),
      - one merged sigmoid over [r|z] (ACT fixed cost dominates, merging
        is nearly free),
      - u = i_n + r*hn written back to PSUM so tanh gets the cheaper
        PSUM-source path,
      - zh = z*h on GpSimd (off the critical chain),
      - s = n*(1-z) as one fused custom DVE op,
      - h' = s + zh in place.
    """
    _gru_mms(nc, cfg, t)
    _gru_ew(nc, pools, cfg, t)


def _gru_mms(nc, cfg, t):
    psA, psB = cfg["psA"], cfg["psB"]
    h_aug = cfg["h_aug"]
    w_ih, w_hh = cfg["w_ih"], cfg["w_hh"]
    rhs_in = cfg["rhs_in"](t)  # [K_in, 128] AP, input incl. ones row
    K_in = rhs_in.shape[0]

    pA = psA.tile([128, 256], F32, tag=cfg["tag"] + "_pA")
    pB = psB.tile([128, 384], F32, tag=cfg["tag"] + "_pB")
    cfg["cur_pA"], cfg["cur_pB"] = pA, pB

    mm = nc.tensor.matmul
    # r gate -> pA[:, 0:128], z gate -> pA[:, 128:256]
    mm(pA[:, 0:128], w_ih[0:K_in, 0:128], rhs_in, start=True, stop=False)
    mm(pA[:, 0:128], w_hh[:, 0:128], h_aug[:], start=False, stop=True)
    mm(pA[:, 128:256], w_ih[0:K_in, 128:256], rhs_in, start=True, stop=False)
    mm(pA[:, 128:256], w_hh[:, 128:256], h_aug[:], start=False, stop=True)
    # i_n (+bih_n) -> pB[:, 0:128]; hn (+bhh_n) -> pB[:, 128:256]
    mm(pB[:, 0:128], w_ih[0:K_in, 256:384], rhs_in, start=True, stop=True)
    mm(pB[:, 128:256], w_hh[:, 256:384], h_aug[:], start=True, stop=True)


def _gru_ew(nc, pools, cfg, t):
    dt = cfg["dt"]
    h_aug = cfg["h_aug"]
    pA, pB = cfg["cur_pA"], cfg["cur_pB"]

    rz = pools["ew"].tile([100, 256], dt, tag=cfg["tag"] + "_rz")
    nc.scalar.activation(rz[:], pA[0:100, :], AF.Sigmoid)

    # zh = z*h (old h) — off the critical chain, on GpSimd
    zh = pools["ew"].tile([100, 128], dt, tag=cfg["tag"] + "_zh")
    nc.gpsimd.tensor_tensor(zh[:], rz[:, 128:256], h_aug[0:100, :], op=OP.mult)

    tt = pools["ew"].tile([100, 128], dt, tag=cfg["tag"] + "_t")
    nc.vector.tensor_tensor(tt[:], rz[:, 0:128], pB[0:100, 128:256], op=OP.mult)
    # u -> PSUM (cols 256:384 of the same bank) so tanh reads PSUM
    nc.vector.tensor_tensor(pB[0:100, 256:384], tt[:], pB[0:100, 0:128],
                            op=OP.add)
    n = pools["ew"].tile([100, 128], dt, tag=cfg["tag"] + "_n")
    nc.scalar.activation(n[:], pB[0:100, 256:384], AF.Tanh)
    s = pools["ew"].tile([100, 128], dt, tag=cfg["tag"] + "_s")
    nc.vector._custom_dve(GRU_NS, out=s[:], in0=n[:], in1=rz[:, 128:256])
    # h' = n*(1-z) + z*h, in place (row 100 = ones)
    nc.vector.tensor_tensor(h_aug[0:100, :], s[:], zh[:], op=OP.add)


def build_bass(T_steps=T, PD_steps=PD):
    nc = bacc.Bacc("TRN2", target_bir_lowering=False, debug=False,
                   num_devices=NCORES)

    xd = nc.dram_tensor("x", [E + 1, T_steps * BL], F16, kind="ExternalInput")
    decd = nc.dram_tensor("dec_in", [E + 1, PD_steps * BL], F32,
                          kind="ExternalInput")
    w_e0_ih = nc.dram_tensor("w_e0_ih", [E + 1, 384], F16, kind="ExternalInput")
    w_e0_hh = nc.dram_tensor("w_e0_hh", [H + 1, 384], F16, kind="ExternalInput")
    w_e1_ih = nc.dram_tensor("w_e1_ih", [H + 1, 384], F16, kind="ExternalInput")
    w_e1_hh = nc.dram_tensor("w_e1_hh", [H + 1, 384], F16, kind="ExternalInput")
    w_d0_ih = nc.dram_tensor("w_d0_ih", [E + 1, 384], F32, kind="ExternalInput")
    w_d0_hh = nc.dram_tensor("w_d0_hh", [H + 1, 384], F32, kind="ExternalInput")
    w_d1_ih = nc.dram_tensor("w_d1_ih", [H + 1, 384], F32, kind="ExternalInput")
    w_d1_hh = nc.dram_tensor("w_d1_hh", [H + 1, 384], F32, kind="ExternalInput")
    w_lin = nc.dram_tensor("w_lin", [H + 1, 64], F32, kind="ExternalInput")
    h16d = nc.dram_tensor("h_init16", [H + 1, BL], F16, kind="ExternalInput")
    h32d = nc.dram_tensor("h_init32", [H + 1, BL], F32, kind="ExternalInput")
    outd = nc.dram_tensor("out", [PD_steps, 64, BL], F32, kind="ExternalOutput")

    with tile.TileContext(nc) as tc:
        from contextlib import ExitStack
        with ExitStack() as ctx:
            const = ctx.enter_context(tc.tile_pool(name="const", bufs=1))
            pools = {
                "ew": ctx.enter_context(tc.tile_pool(name="ew", bufs=3)),
                "out": ctx.enter_context(tc.tile_pool(name="outp", bufs=4)),
            }

            # ---- persistent SBUF residents ----
            dec_all = const.tile([E + 1, PD_steps * BL], F32, tag="dec_all")
            we0i = const.tile([E + 1, 384], F16, tag="we0i")
            we0h = const.tile([H + 1, 384], F16, tag="we0h")
            we1i = const.tile([H + 1, 384], F16, tag="we1i")
            we1h = const.tile([H + 1, 384], F16, tag="we1h")
            wd0i = const.tile([E + 1, 384], F32, tag="wd0i")
            wd0h = const.tile([H + 1, 384], F32, tag="wd0h")
            wd1i = const.tile([H + 1, 384], F32, tag="wd1i")
            wd1h = const.tile([H + 1, 384], F32, tag="wd1h")
            wlin = const.tile([H + 1, 64], F32, tag="wlin")
            h_e0 = const.tile([H + 1, BL], F16, tag="h_e0")
            h_e1 = const.tile([H + 1, BL], F16, tag="h_e1")
            h_d0 = const.tile([H + 1, BL], F32, tag="h_d0")
            h_d1 = const.tile([H + 1, BL], F32, tag="h_d1")

            # ---- loads ----
            nc.sync.dma_start(dec_all[:], decd[:])
            for sb, dr in ((we0i, w_e0_ih), (we0h, w_e0_hh), (we1i, w_e1_ih),
                           (we1h, w_e1_hh), (wd0i, w_d0_ih), (wd0h, w_d0_hh),
                           (wd1i, w_d1_ih), (wd1h, w_d1_hh), (wlin, w_lin)):
                nc.sync.dma_start(sb[:], dr[:])

            for ht, src in ((h_e0, h16d), (h_e1, h16d), (h_d0, h32d),
                            (h_d1, h32d)):
                nc.sync.dma_start(ht[:], src[:])

            # ---- encoder ----
            # x streams through a double-buffered chunk pool (the full
            # [27, 64000] fp16 slab would eat 125KB/partition of SBUF).
            XCH = 50  # steps per chunk
            with ExitStack() as enc_ctx:
                psA = enc_ctx.enter_context(
                    tc.tile_pool(name="psA", bufs=2, space="PSUM"))
                psB = enc_ctx.enter_context(
                    tc.tile_pool(name="psB", bufs=2, space="PSUM"))
                xpool = enc_ctx.enter_context(
                    tc.tile_pool(name="xpool", bufs=2))
                xchunk = [None]

                def x_rhs(t):
                    return xchunk[0][:, (t % XCH) * BL:(t % XCH + 1) * BL]

                cfg0 = dict(psA=psA, psB=psB, wpool=None, dt=F16, h_aug=h_e0,
                            w_ih=we0i, w_hh=we0h, tag="e0", gp_d=True,
                            rhs_in=x_rhs)
                cfg1 = dict(psA=psA, psB=psB, wpool=None, dt=F16, h_aug=h_e1,
                            w_ih=we1i, w_hh=we1h, tag="e1", gp_d=True,
                            rhs_in=lambda t: h_e0[:])
                # Emission order per slot: all matmuls first (l0's lead so
                # the PE queue unblocks on l0's h' alone), then elementwise.
                # l1's matmuls read h_e0 (= e0[k-1]) and are traced before
                # l0's in-place h' overwrite — Tile's WAR dep keeps order.
                for k in range(T_steps + 1):
                    if k >= 1:
                        _gru_mms(nc, cfg1, k - 1)
                        _gru_ew(nc, pools, cfg1, k - 1)
                    if k < T_steps:
                        if k % XCH == 0:
                            nsteps = min(XCH, T_steps - k)
                            xc = xpool.tile([E + 1, XCH * BL], F16, tag="xc")
                            nc.sync.dma_start(
                                xc[:, 0:nsteps * BL],
                                xd[:, k * BL:(k + nsteps) * BL])
                            xchunk[0] = xc
                        _gru_mms(nc, cfg0, k)
                        _gru_ew(nc, pools, cfg0, k)

            # hand encoder final states to the decoder (fp16 -> fp32)
            nc.vector.tensor_copy(h_d0[0:H, :], h_e0[0:H, :])
            nc.vector.tensor_copy(h_d1[0:H, :], h_e1[0:H, :])

            # ---- decoder ----
            with ExitStack() as dec_ctx:
                psAd = dec_ctx.enter_context(
                    tc.tile_pool(name="psAd", bufs=2, space="PSUM"))
                psBd = dec_ctx.enter_context(
                    tc.tile_pool(name="psBd", bufs=1, space="PSUM"))
                psL = dec_ctx.enter_context(
                    tc.tile_pool(name="psL", bufs=2, space="PSUM"))
                cfgd0 = dict(psA=psAd, psB=psBd, wpool=None, dt=F32, h_aug=h_d0,
                             w_ih=wd0i, w_hh=wd0h, tag="d0", gp_d=True,
                             rhs_in=lambda t: dec_all[:, t * BL:(t + 1) * BL])
                cfgd1 = dict(psA=psAd, psB=psBd, wpool=None, dt=F32, h_aug=h_d1,
                             w_ih=wd1i, w_hh=wd1h, tag="d1", gp_d=True,
                             rhs_in=lambda t: h_d0[:])
                for k in range(PD_steps + 1):
                    if k >= 1:
                        _gru_mms(nc, cfgd1, k - 1)
                        _gru_ew(nc, pools, cfgd1, k - 1)
                        j = k - 1
                        pL = psL.tile([64, BL], F32, tag="pL")
                        nc.tensor.matmul(pL[:], wlin[:, 0:64], h_d1[:],
                                         start=True, stop=True)
                        ob = pools["out"].tile([64, BL], F32, tag="ob")
                        nc.scalar.activation(ob[:], pL[:], AF.Copy)
                        nc.sync.dma_start(outd[j], ob[:])
                    if k < PD_steps:
                        _gru_mms(nc, cfgd0, k)
                        _gru_ew(nc, pools, cfgd0, k)

    nc.compile()
    return nc


def _build_inputs(inputs, T_steps=T, PD_steps=PD):
    x = np.asarray(inputs["x"], np.float32)
    target = np.asarray(inputs["target"])
    emb = np.asarray(inputs["emb"], np.float32)
    dec_in_full = emb[target[:PD_steps, :, 0].astype(np.int64)]  # [PD, B, E]

    def g(name):
        return np.asarray(inputs[name], np.float32)

    h16 = np.zeros((H + 1, BL), np.float16)
    h16[H, :] = 1.0
    h32 = np.zeros((H + 1, BL), np.float32)
    h32[H, :] = 1.0
    shared = {
        "w_e0_ih": _pack_w(g("enc_Wih0"), g("enc_bih0"), np.float16),
        "w_e0_hh": _pack_w(g("enc_Whh0"), g("enc_bhh0"), np.float16),
        "w_e1_ih": _pack_w(g("enc_Wih1"), g("enc_bih1"), np.float16),
        "w_e1_hh": _pack_w(g("enc_Whh1"), g("enc_bhh1"), np.float16),
        "w_d0_ih": _pack_w(g("dec_Wih0"), g("dec_bih0"), np.float32),
        "w_d0_hh": _pack_w(g("dec_Whh0"), g("dec_bhh0"), np.float32),
        "w_d1_ih": _pack_w(g("dec_Wih1"), g("dec_bih1"), np.float32),
        "w_d1_hh": _pack_w(g("dec_Whh1"), g("dec_bhh1"), np.float32),
        # [lin_W.T ; lin_b] -> [H+1, 64], ones row of rhs hits the bias row
        "w_lin": np.concatenate(
            [g("lin_W").T, g("lin_b")[None, :]], axis=0).astype(np.float32),
        "h_init16": h16,
        "h_init32": h32,
    }
    in_maps = []
    for c in range(NCORES):
        bs = slice(c * BL, (c + 1) * BL)
        m = dict(shared)
        m["x"] = _prep_seq(x[:T_steps, bs, :], np.float16)
        m["dec_in"] = _prep_seq(dec_in_full[:, bs, :], np.float32)
        in_maps.append(m)
    return in_maps, target


def _install_trace_shim():
    """Make run_bass_kernel_spmd(trace=True) work in this image: provide the
    missing antenv.axon_hooks module and neuter the S3 artifact upload."""
    import types, sys as _sys
    if "antenv.axon_hooks" not in _sys.modules:
        mod = types.ModuleType("antenv.axon_hooks")
        mod._hook = None
        mod.set_axon_ntff_profile_hook = lambda h: setattr(mod, "_hook", h)
        mod.get_axon_ntff_profile_hook = lambda: mod._hook
        _sys.modules["antenv.axon_hooks"] = mod
    from antenv.axon_hooks import (get_axon_ntff_profile_hook,
                                   set_axon_ntff_profile_hook)
    if get_axon_ntff_profile_hook() is None:
        _sys.path.insert(0, "/root/.axon_site")
        from trn_agent_boot.trn_boot import _ntff_profile_via_ctypes
        set_axon_ntff_profile_hook(
            _ntff_profile_via_ctypes("/opt/axon/libaxon_pjrt.so"))
    from concourse import bass_utils as _bu
    _bu.upload_artifacts = lambda tmpdir: f"local:{tmpdir}"


def run_device(inputs, T_steps=T, PD_steps=PD, trace=False):
    if trace:
        _install_trace_shim()
    key = (T_steps, PD_steps)
    if key not in _CACHE:
        _CACHE[key] = build_bass(T_steps, PD_steps)
    nc = _CACHE[key]
    in_maps, target = _build_inputs(inputs, T_steps, PD_steps)
    res = run_bass_kernel_spmd(nc, in_maps, list(range(NCORES)), trace=trace)
    # out per core: [PD, 64, BL] logitsT -> [PD, BL, 64]
    logits = np.concatenate(
        [res.results[c]["out"].transpose(0, 2, 1) for c in range(NCORES)],
        axis=1)  # [PD, B, 64]
    return logits, target, res


def kernel(**inputs):
    logits, target, _ = run_device(inputs)
    softmax_cal = logits.reshape(-1, D).astype(np.float32)
    target = np.asarray(target)
    target_cal = target[1:].reshape(-1)
    # jnp.argmax yields int32 under x64-off, int64 under x64-on; the target
    # input dtype tells us which config the harness reference ran under.
    idx_dt = np.int64 if target.dtype == np.int64 else np.int32
    asr_outputs = np.argmax(logits, axis=2)[:, :, None].astype(idx_dt)
    return softmax_cal, target_cal, asr_outputs


# revision 24
# speedup vs baseline: 1.0615x; 1.0615x over previous
"""Trainium2 Bass kernel for nn_AsrModel (2-layer GRU encoder, 2-layer GRU
teacher-forced decoder, linear head + argmax).

Strategy:
  - Data-parallel over batch: B=1024 sharded 8 ways -> B_local=128 per core.
  - Hidden-major layout on chip: state tiles are [1+H, B_local] = [101, 128]
    with row 0 == 1.0 (the "ones row") so biases ride in as an extra
    contraction row of the gate matmuls.
  - Encoder (500 steps) runs in fp16 (weights/state/gates; PSUM fp32):
    fp16 matmul is 1 cyc/row vs fp32's 4, and DVE elementwise gets 2x mode.
  - Decoder (79 steps) runs in fp32 to keep logits accurate enough that the
    argmax output matches the reference (validated on host: fp16 encoder +
    fp32 decoder -> zero argmax flips, logits rel err ~1e-4).
  - The two layers of each phase are staggered by one time step - two
    independent dependency chains that hide per-op latency.
  - Per layer-step: 6 matmuls into PSUM (ih+hh per gate, biases via ones
    row), one merged Sigmoid over [100,256] (r|z), DVE gate arithmetic,
    Tanh, and an in-place h update.  h - n runs on GpSimd to offload DVE.
  - Final linear: lhsT = [lin_b; lin_W.T] (constant stationary), rhs = the
    decoder-l1 state tile -> logitsT [64, 128] in PSUM, copied out via
    ScalarE and DMA'd per step.
  - Host side: shard/transpose/pack inputs, embedding gather, final
    argmax + reshape (cheap numpy; HW time is the NEFF execution).
"""

import sys

sys.path.insert(0, "/opt/trn_rl_repo")

import numpy as np

from concourse import bass, bacc, tile
import concourse.mybir as mybir
from concourse.bass_utils import run_bass_kernel_spmd
from concourse import dve_ops
from concourse.dve_spec import Spec, Src0, Src1, lower as dve_lower
from concourse.dve_uop import DveOpSpec


def _register_gru_ns():
    """Custom fused DVE op: out = in0 - in0*in1 = n*(1-z).
    Saves one tensor_tensor on the critical h-update path."""
    if "GRU_NS" in dve_ops._SUB_OPCODE_FOR_NAME:
        return dve_ops._SUB_OPCODE_FOR_NAME and [
            o for o in dve_ops.OPS if o.name == "GRU_NS"][0]
    spec = Spec(
        body=Src0 - Src0 * Src1,
        reference=lambda in0, in1, s0, s1, imm2: in0.astype(np.float32)
        - in0.astype(np.float32) * in1.astype(np.float32),
    )
    opcode = dve_ops._CUSTOM_DVE_ROW_BASE + len(dve_ops.OPS)
    shas = {}
    for ver in ("v3", "v4"):
        uops = dve_lower(spec, ver=ver)
        shas[ver] = DveOpSpec(
            name="GRU_NS", opcode=opcode, uops=uops,
            rd1_en=True).sha(ver)
    op = dve_ops.DveOp("GRU_NS", spec, subdim=False, uops_sha=shas)
    dve_ops.OPS.append(op)
    dve_ops._SUB_OPCODE_FOR_NAME["GRU_NS"] = opcode
    return op


GRU_NS = _register_gru_ns()

T, B, E, H, P, D = 500, 1024, 26, 100, 80, 64
NCORES = 8
BL = B // NCORES  # 128
PD = P - 1  # decoder steps actually needed (logits[:-1])

F16 = mybir.dt.float16
F32 = mybir.dt.float32
AF = mybir.ActivationFunctionType
OP = mybir.AluOpType

USE_GPSIMD = True

_CACHE = {}


def _pack_w(Wih, bih, np_dt):
    """lhsT for gate matmuls, one [K,128] block per gate packed along the
    free dim -> [K_in+1, 384].  Rows = [W.T ; bias] matching rhs layouts
    [x ; ones] / [h ; ones] — the ones row is always LAST (engine access
    patterns must start at partition 0)."""
    K_in = Wih.shape[1]
    out = np.zeros((K_in + 1, 3 * 128), np.float32)
    for g in range(3):
        out[:K_in, g * 128:g * 128 + H] = Wih[g * H:(g + 1) * H, :].T
        out[K_in, g * 128:g * 128 + H] = bih[g * H:(g + 1) * H]
    return out.astype(np_dt)


def _prep_seq(x_tbe, np_dt):
    """[T, BL, E] -> [E+1, T*BL] with last row = 1.0 (ones row)."""
    Tn = x_tbe.shape[0]
    out = np.ones((E + 1, Tn * BL), np.float32)
    out[:E, :] = x_tbe.transpose(2, 0, 1).reshape(E, Tn * BL)
    return out.astype(np_dt)


def _gru_layer_steps(nc, tc, pools, cfg, t):
    """Emit one step of one GRU layer.

    h' = n*(1-z) + z*h with:
      - gate pre-activations via PSUM matmuls (biases ride the ones row),
      - one merged sigmoid over [r|z] (ACT fixed cost dominates, merging
        is nearly free),
      - u = i_n + r*hn written back to PSUM so tanh gets the cheaper
        PSUM-source path,
      - zh = z*h on GpSimd (off the critical chain),
      - s = n*(1-z) as one fused custom DVE op,
      - h' = s + zh in place.
    """
    _gru_mms(nc, cfg, t)
    _gru_ew(nc, pools, cfg, t)


def _gru_mms(nc, cfg, t):
    psA, psB = cfg["psA"], cfg["psB"]
    h_aug = cfg["h_aug"]
    w_ih, w_hh = cfg["w_ih"], cfg["w_hh"]
    rhs_in = cfg["rhs_in"](t)  # [K_in, 128] AP, input incl. ones row
    K_in = rhs_in.shape[0]

    pA = psA.tile([128, 256], F32, tag=cfg["tag"] + "_pA")
    pB = psB.tile([128, 384], F32, tag=cfg["tag"] + "_pB")
    cfg["cur_pA"], cfg["cur_pB"] = pA, pB

    mm = nc.tensor.matmul
    # r gate -> pA[:, 0:128], z gate -> pA[:, 128:256].  r's two matmuls go
    # FIRST so sigma_r's PE-sem threshold unblocks after just two matmuls.
    mm(pA[:, 0:128], w_ih[0:K_in, 0:128], rhs_in, start=True, stop=False)
    mm(pA[:, 0:128], w_hh[:, 0:128], h_aug[:], start=False, stop=True)
    # sigma_r immediately — the only activation on the critical chain
    dt = cfg["dt"]
    rz = cfg["pools"]["ew"].tile([100, 256], dt, tag=cfg["tag"] + "_rz")
    cfg["cur_rz"] = rz
    nc.scalar.activation(rz[:, 0:128], pA[0:100, 0:128], AF.Sigmoid)
    mm(pA[:, 128:256], w_ih[0:K_in, 128:256], rhs_in, start=True, stop=False)
    mm(pA[:, 128:256], w_hh[:, 128:256], h_aug[:], start=False, stop=True)
    # i_n (+bih_n) -> pB[:, 0:128]; hn (+bhh_n) -> pB[:, 128:256]
    mm(pB[:, 0:128], w_ih[0:K_in, 256:384], rhs_in, start=True, stop=True)
    mm(pB[:, 128:256], w_hh[:, 256:384], h_aug[:], start=True, stop=True)
    nc.scalar.activation(rz[:, 128:256], pA[0:100, 128:256], AF.Sigmoid)


def _gru_ew(nc, pools, cfg, t):
    dt = cfg["dt"]
    h_aug = cfg["h_aug"]
    pA, pB = cfg["cur_pA"], cfg["cur_pB"]
    rz = cfg["cur_rz"]

    # zh = z*h (old h) — off the critical chain, on GpSimd
    zh = pools["ew"].tile([100, 128], dt, tag=cfg["tag"] + "_zh")
    nc.gpsimd.tensor_tensor(zh[:], rz[:, 128:256], h_aug[0:100, :], op=OP.mult)

    tt = pools["ew"].tile([100, 128], dt, tag=cfg["tag"] + "_t")
    nc.vector.tensor_tensor(tt[:], rz[:, 0:128], pB[0:100, 128:256], op=OP.mult)
    # u -> PSUM (cols 256:384 of the same bank) so tanh reads PSUM
    nc.vector.tensor_tensor(pB[0:100, 256:384], tt[:], pB[0:100, 0:128],
                            op=OP.add)
    n = pools["ew"].tile([100, 128], dt, tag=cfg["tag"] + "_n")
    nc.scalar.activation(n[:], pB[0:100, 256:384], AF.Tanh)
    s = pools["ew"].tile([100, 128], dt, tag=cfg["tag"] + "_s")
    nc.vector._custom_dve(GRU_NS, out=s[:], in0=n[:], in1=rz[:, 128:256])
    # h' = n*(1-z) + z*h, in place (row 100 = ones)
    nc.vector.tensor_tensor(h_aug[0:100, :], s[:], zh[:], op=OP.add)


def build_bass(T_steps=T, PD_steps=PD):
    nc = bacc.Bacc("TRN2", target_bir_lowering=False, debug=False,
                   num_devices=NCORES)

    xd = nc.dram_tensor("x", [E + 1, T_steps * BL], F16, kind="ExternalInput")
    decd = nc.dram_tensor("dec_in", [E + 1, PD_steps * BL], F32,
                          kind="ExternalInput")
    w_e0_ih = nc.dram_tensor("w_e0_ih", [E + 1, 384], F16, kind="ExternalInput")
    w_e0_hh = nc.dram_tensor("w_e0_hh", [H + 1, 384], F16, kind="ExternalInput")
    w_e1_ih = nc.dram_tensor("w_e1_ih", [H + 1, 384], F16, kind="ExternalInput")
    w_e1_hh = nc.dram_tensor("w_e1_hh", [H + 1, 384], F16, kind="ExternalInput")
    w_d0_ih = nc.dram_tensor("w_d0_ih", [E + 1, 384], F32, kind="ExternalInput")
    w_d0_hh = nc.dram_tensor("w_d0_hh", [H + 1, 384], F32, kind="ExternalInput")
    w_d1_ih = nc.dram_tensor("w_d1_ih", [H + 1, 384], F32, kind="ExternalInput")
    w_d1_hh = nc.dram_tensor("w_d1_hh", [H + 1, 384], F32, kind="ExternalInput")
    w_lin = nc.dram_tensor("w_lin", [H + 1, 64], F32, kind="ExternalInput")
    h16d = nc.dram_tensor("h_init16", [H + 1, BL], F16, kind="ExternalInput")
    h32d = nc.dram_tensor("h_init32", [H + 1, BL], F32, kind="ExternalInput")
    outd = nc.dram_tensor("out", [PD_steps, 64, BL], F32, kind="ExternalOutput")

    with tile.TileContext(nc) as tc:
        from contextlib import ExitStack
        with ExitStack() as ctx:
            const = ctx.enter_context(tc.tile_pool(name="const", bufs=1))
            pools = {
                "ew": ctx.enter_context(tc.tile_pool(name="ew", bufs=3)),
                "out": ctx.enter_context(tc.tile_pool(name="outp", bufs=4)),
            }

            # ---- persistent SBUF residents ----
            dec_all = const.tile([E + 1, PD_steps * BL], F32, tag="dec_all")
            we0i = const.tile([E + 1, 384], F16, tag="we0i")
            we0h = const.tile([H + 1, 384], F16, tag="we0h")
            we1i = const.tile([H + 1, 384], F16, tag="we1i")
            we1h = const.tile([H + 1, 384], F16, tag="we1h")
            wd0i = const.tile([E + 1, 384], F32, tag="wd0i")
            wd0h = const.tile([H + 1, 384], F32, tag="wd0h")
            wd1i = const.tile([H + 1, 384], F32, tag="wd1i")
            wd1h = const.tile([H + 1, 384], F32, tag="wd1h")
            wlin = const.tile([H + 1, 64], F32, tag="wlin")
            h_e0 = const.tile([H + 1, BL], F16, tag="h_e0")
            h_e1 = const.tile([H + 1, BL], F16, tag="h_e1")
            h_d0 = const.tile([H + 1, BL], F32, tag="h_d0")
            h_d1 = const.tile([H + 1, BL], F32, tag="h_d1")

            # ---- loads ----
            nc.sync.dma_start(dec_all[:], decd[:])
            for sb, dr in ((we0i, w_e0_ih), (we0h, w_e0_hh), (we1i, w_e1_ih),
                           (we1h, w_e1_hh), (wd0i, w_d0_ih), (wd0h, w_d0_hh),
                           (wd1i, w_d1_ih), (wd1h, w_d1_hh), (wlin, w_lin)):
                nc.sync.dma_start(sb[:], dr[:])

            for ht, src in ((h_e0, h16d), (h_e1, h16d), (h_d0, h32d),
                            (h_d1, h32d)):
                nc.sync.dma_start(ht[:], src[:])

            # ---- encoder ----
            # x streams through a double-buffered chunk pool (the full
            # [27, 64000] fp16 slab would eat 125KB/partition of SBUF).
            XCH = 50  # steps per chunk
            with ExitStack() as enc_ctx:
                psA = enc_ctx.enter_context(
                    tc.tile_pool(name="psA", bufs=2, space="PSUM"))
                psB = enc_ctx.enter_context(
                    tc.tile_pool(name="psB", bufs=2, space="PSUM"))
                xpool = enc_ctx.enter_context(
                    tc.tile_pool(name="xpool", bufs=2))
                xchunk = [None]

                def x_rhs(t):
                    return xchunk[0][:, (t % XCH) * BL:(t % XCH + 1) * BL]

                cfg0 = dict(psA=psA, psB=psB, pools=pools, dt=F16, h_aug=h_e0,
                            w_ih=we0i, w_hh=we0h, tag="e0", gp_d=True,
                            rhs_in=x_rhs)
                cfg1 = dict(psA=psA, psB=psB, pools=pools, dt=F16, h_aug=h_e1,
                            w_ih=we1i, w_hh=we1h, tag="e1", gp_d=True,
                            rhs_in=lambda t: h_e0[:])
                # Emission order per slot: all matmuls first (l0's lead so
                # the PE queue unblocks on l0's h' alone), then elementwise.
                # l1's matmuls read h_e0 (= e0[k-1]) and are traced before
                # l0's in-place h' overwrite — Tile's WAR dep keeps order.
                for k in range(T_steps + 1):
                    if k >= 1:
                        _gru_mms(nc, cfg1, k - 1)
                        _gru_ew(nc, pools, cfg1, k - 1)
                    if k < T_steps:
                        if k % XCH == 0:
                            nsteps = min(XCH, T_steps - k)
                            xc = xpool.tile([E + 1, XCH * BL], F16, tag="xc")
                            nc.sync.dma_start(
                                xc[:, 0:nsteps * BL],
                                xd[:, k * BL:(k + nsteps) * BL])
                            xchunk[0] = xc
                        _gru_mms(nc, cfg0, k)
                        _gru_ew(nc, pools, cfg0, k)

            # hand encoder final states to the decoder (fp16 -> fp32)
            nc.vector.tensor_copy(h_d0[0:H, :], h_e0[0:H, :])
            nc.vector.tensor_copy(h_d1[0:H, :], h_e1[0:H, :])

            # ---- decoder ----
            with ExitStack() as dec_ctx:
                psAd = dec_ctx.enter_context(
                    tc.tile_pool(name="psAd", bufs=2, space="PSUM"))
                psBd = dec_ctx.enter_context(
                    tc.tile_pool(name="psBd", bufs=1, space="PSUM"))
                psL = dec_ctx.enter_context(
                    tc.tile_pool(name="psL", bufs=2, space="PSUM"))
                cfgd0 = dict(psA=psAd, psB=psBd, pools=pools, dt=F32, h_aug=h_d0,
                             w_ih=wd0i, w_hh=wd0h, tag="d0", gp_d=True,
                             rhs_in=lambda t: dec_all[:, t * BL:(t + 1) * BL])
                cfgd1 = dict(psA=psAd, psB=psBd, pools=pools, dt=F32, h_aug=h_d1,
                             w_ih=wd1i, w_hh=wd1h, tag="d1", gp_d=True,
                             rhs_in=lambda t: h_d0[:])
                for k in range(PD_steps + 1):
                    if k >= 1:
                        _gru_mms(nc, cfgd1, k - 1)
                        _gru_ew(nc, pools, cfgd1, k - 1)
                        j = k - 1
                        pL = psL.tile([64, BL], F32, tag="pL")
                        nc.tensor.matmul(pL[:], wlin[:, 0:64], h_d1[:],
                                         start=True, stop=True)
                        ob = pools["out"].tile([64, BL], F32, tag="ob")
                        nc.scalar.activation(ob[:], pL[:], AF.Copy)
                        nc.sync.dma_start(outd[j], ob[:])
                    if k < PD_steps:
                        _gru_mms(nc, cfgd0, k)
                        _gru_ew(nc, pools, cfgd0, k)

    nc.compile()
    return nc


def _build_inputs(inputs, T_steps=T, PD_steps=PD):
    x = np.asarray(inputs["x"], np.float32)
    target = np.asarray(inputs["target"])
    emb = np.asarray(inputs["emb"], np.float32)
    dec_in_full = emb[target[:PD_steps, :, 0].astype(np.int64)]  # [PD, B, E]

    def g(name):
        return np.asarray(inputs[name], np.float32)

    h16 = np.zeros((H + 1, BL), np.float16)
    h16[H, :] = 1.0
    h32 = np.zeros((H + 1, BL), np.float32)
    h32[H, :] = 1.0
    shared = {
        "w_e0_ih": _pack_w(g("enc_Wih0"), g("enc_bih0"), np.float16),
        "w_e0_hh": _pack_w(g("enc_Whh0"), g("enc_bhh0"), np.float16),
        "w_e1_ih": _pack_w(g("enc_Wih1"), g("enc_bih1"), np.float16),
        "w_e1_hh": _pack_w(g("enc_Whh1"), g("enc_bhh1"), np.float16),
        "w_d0_ih": _pack_w(g("dec_Wih0"), g("dec_bih0"), np.float32),
        "w_d0_hh": _pack_w(g("dec_Whh0"), g("dec_bhh0"), np.float32),
        "w_d1_ih": _pack_w(g("dec_Wih1"), g("dec_bih1"), np.float32),
        "w_d1_hh": _pack_w(g("dec_Whh1"), g("dec_bhh1"), np.float32),
        # [lin_W.T ; lin_b] -> [H+1, 64], ones row of rhs hits the bias row
        "w_lin": np.concatenate(
            [g("lin_W").T, g("lin_b")[None, :]], axis=0).astype(np.float32),
        "h_init16": h16,
        "h_init32": h32,
    }
    in_maps = []
    for c in range(NCORES):
        bs = slice(c * BL, (c + 1) * BL)
        m = dict(shared)
        m["x"] = _prep_seq(x[:T_steps, bs, :], np.float16)
        m["dec_in"] = _prep_seq(dec_in_full[:, bs, :], np.float32)
        in_maps.append(m)
    return in_maps, target


def _install_trace_shim():
    """Make run_bass_kernel_spmd(trace=True) work in this image: provide the
    missing antenv.axon_hooks module and neuter the S3 artifact upload."""
    import types, sys as _sys
    if "antenv.axon_hooks" not in _sys.modules:
        mod = types.ModuleType("antenv.axon_hooks")
        mod._hook = None
        mod.set_axon_ntff_profile_hook = lambda h: setattr(mod, "_hook", h)
        mod.get_axon_ntff_profile_hook = lambda: mod._hook
        _sys.modules["antenv.axon_hooks"] = mod
    from antenv.axon_hooks import (get_axon_ntff_profile_hook,
                                   set_axon_ntff_profile_hook)
    if get_axon_ntff_profile_hook() is None:
        _sys.path.insert(0, "/root/.axon_site")
        from trn_agent_boot.trn_boot import _ntff_profile_via_ctypes
        set_axon_ntff_profile_hook(
            _ntff_profile_via_ctypes("/opt/axon/libaxon_pjrt.so"))
    from concourse import bass_utils as _bu
    _bu.upload_artifacts = lambda tmpdir: f"local:{tmpdir}"


def run_device(inputs, T_steps=T, PD_steps=PD, trace=False):
    if trace:
        _install_trace_shim()
    key = (T_steps, PD_steps)
    if key not in _CACHE:
        _CACHE[key] = build_bass(T_steps, PD_steps)
    nc = _CACHE[key]
    in_maps, target = _build_inputs(inputs, T_steps, PD_steps)
    res = run_bass_kernel_spmd(nc, in_maps, list(range(NCORES)), trace=trace)
    # out per core: [PD, 64, BL] logitsT -> [PD, BL, 64]
    logits = np.concatenate(
        [res.results[c]["out"].transpose(0, 2, 1) for c in range(NCORES)],
        axis=1)  # [PD, B, 64]
    return logits, target, res


def kernel(**inputs):
    logits, target, _ = run_device(inputs)
    softmax_cal = logits.reshape(-1, D).astype(np.float32)
    target = np.asarray(target)
    target_cal = target[1:].reshape(-1)
    # jnp.argmax yields int32 under x64-off, int64 under x64-on; the target
    # input dtype tells us which config the harness reference ran under.
    idx_dt = np.int64 if target.dtype == np.int64 else np.int32
    asr_outputs = np.argmax(logits, axis=2)[:, :, None].astype(idx_dt)
    return softmax_cal, target_cal, asr_outputs
